# revision 15
# baseline (speedup 1.0000x reference)
"""MixAdapter: alpha-weighted adapter superposition + joint layernorm + bottleneck MLP.

Two SPMD launches on 8 NeuronCores:

  Launch A ("merge"): the adapter stacks are converted to fp16 on host and
    sharded across the 8 cores (~5MB each); each core computes its slice of
    the alpha-weighted merged parameters with a fused (W*alpha)+acc DVE chain
    (fp16 operands -> DVE 2x mode).  Host gathers the 0.8MB of merged params.

  Host folding (between launches, all tiny):
    wdTw = W_ln[d] * W_down[o,d] scaled and quantized to fp8e4 (DoubleRow
    operand), wuT zero-padded to 512 rows / scaled / quantized to fp8e4,
    P = W_down@b_ln and Q = W_down@W_ln vectors for the ReLU bias
    (bias_h = P - mu*rstd*Q, with mu/rstd computed on device).

  Launch B ("main"): data-parallel over batch (batch elem k -> core k).
    x^T arrives as fp16; the scalar engine downcasts it to fp8e4 (x*32)
    while accumulating per-partition sums (S1); DVE computes sum-of-squares
    via fused tensor_tensor_reduce.  Down-proj and up-proj run as fp8e4
    DoubleRow matmuls (2 contraction rows per PE pass).  ReLU folds
    rstd/bias and re-quantizes h to fp8e4.  Residual add reads the exact
    fp16 x tiles; y is written fp16 and upcast on host.
"""

import numpy as np
import ml_dtypes

from concourse import bacc, mybir, tile
import concourse.bass as bass
from concourse.bass_utils import run_bass_kernel_spmd

# Problem shapes (hardcoded per the task contract).
B, S, D, BOT, N = 8, 2048, 1024, 400, 25
NCORES = 8
EPS = 1e-5
FP32 = mybir.dt.float32
F16 = mybir.dt.float16
F8 = mybir.dt.float8e4
NP_F8 = ml_dtypes.float8_e4m3
F8_MAX = 240.0

DC = D // 128        # 8 d-chunks
OC = 4               # o-chunks (400 -> 3x128 + 16; padded to 512 for up-proj)
O_SZ = [128, 128, 128, 16]
NSB = S // 256       # 8 moving blocks of 256 (DoubleRow moving limit)
NSBP = S // 512      # 4 psum bank groups of 512

# fp8 scale factors (powers of two; folded back out exactly).
X_SCL = 32.0
W_SCL = 4096.0
WU_SCL = 1024.0
H_SCL = 64.0
PSD_INV = 1.0 / (W_SCL * X_SCL)          # down psum -> z units
PSU_INV = 1.0 / (WU_SCL * H_SCL)         # up psum -> adapter units

USE_F32R = False  # kept for test.py compatibility

# Per-core slice sizes for the merge launch.
WD_ROWS = BOT // NCORES          # 50 rows of W_down per core
WU_ROWS = D // NCORES            # 128 rows of W_up per core
MF = 400 + 400 + 2 * DC          # packed free size per adapter: wd | wu | ln

DR = mybir.MatmulPerfMode.DoubleRow


# ---------------------------------------------------------------------------
# Launch A: alpha-weighted merge of the adapter stacks (sharded over cores)
# ---------------------------------------------------------------------------

def build_merge_nc():
    nc = bacc.Bacc("TRN2", target_bir_lowering=False, debug=False,
                   enable_asserts=False, num_devices=NCORES)

    stack = nc.dram_tensor("stack", [N, 128, MF], F16, kind="ExternalInput")
    alphas = nc.dram_tensor("alphas", [1, N], FP32, kind="ExternalInput")
    out_m = nc.dram_tensor("out_m", [128, MF], F16, kind="ExternalOutput")

    with tile.TileContext(nc) as tc:
        with (
            tc.tile_pool(name="consts", bufs=1) as consts,
            tc.tile_pool(name="acc", bufs=1) as accp,
            tc.tile_pool(name="stk", bufs=6) as stk_pool,
            tc.tile_pool(name="psum", bufs=1, space="PSUM") as psum,
        ):
            # Broadcast alphas across partitions: [1,25] -> [128,25] via PE.
            a_sb = consts.tile([1, N], FP32)
            nc.sync.dma_start(a_sb[:], alphas[:])
            ones_row = consts.tile([1, 128], FP32)
            nc.vector.memset(ones_row[:], 1.0)
            pa = psum.tile([128, N], FP32)
            nc.tensor.matmul(pa[:], ones_row[:], a_sb[:], start=True, stop=True)
            a_bc = consts.tile([128, N], FP32)
            nc.scalar.copy(a_bc[:], pa[:])

            # ts_mul runs in the DVE 4x perf mode and tensor_tensor in 2x;
            # the fused scalar_tensor_tensor would be 1x — slower overall.
            acc = accp.tile([128, MF], F16)
            for n in range(N):
                st = stk_pool.tile([128, MF], F16)
                nc.sync.dma_start(st[:], stack[n])
                al = a_bc[:, n:n + 1]
                if n == 0:
                    nc.vector.tensor_scalar_mul(acc[:], st[:], al)
                else:
                    tm = stk_pool.tile([128, MF], F16, name=f"tm{n}", tag="tm")
                    nc.vector.tensor_scalar_mul(tm[:], st[:], al)
                    nc.vector.tensor_tensor(acc[:], acc[:], tm[:],
                                            mybir.AluOpType.add)

            nc.sync.dma_start(out_m[:], acc[:])

    nc.finalize()
    return nc


# ---------------------------------------------------------------------------
# Launch B: layernorm + down/up projections, one batch element per core
# ---------------------------------------------------------------------------

def build_main_nc():
    nc = bacc.Bacc("TRN2", target_bir_lowering=False, debug=False,
                   enable_asserts=False, num_devices=NCORES)

    xT16 = nc.dram_tensor("xT16", [128, DC, S], F16, kind="ExternalInput")
    # fp8 weights cross the PJRT boundary as uint8; bitcast to fp8 on device.
    wd8 = nc.dram_tensor("wd8", [128, DC, BOT], mybir.dt.uint8, kind="ExternalInput")
    wu8 = nc.dram_tensor("wu8", [128, OC, D], mybir.dt.uint8, kind="ExternalInput")
    pq = nc.dram_tensor("pq", [128, 2 * OC], FP32, kind="ExternalInput")
    yT = nc.dram_tensor("yT", [128, NSBP, DC, 512], F16, kind="ExternalOutput")

    inv1 = 1.0 / (X_SCL * float(S * D))   # S1 -> mu
    inv2 = 1.0 / float(S * D)             # S2 -> E[x^2]

    with tile.TileContext(nc) as tc:
        with (
            tc.tile_pool(name="xt", bufs=1) as xt_pool,
            tc.tile_pool(name="x8", bufs=1) as x8_pool,
            tc.tile_pool(name="ht", bufs=1) as ht_pool,
            tc.tile_pool(name="w", bufs=1) as w_pool,
            tc.tile_pool(name="small", bufs=1) as small,
            tc.tile_pool(name="sq", bufs=3) as sq_pool,
            tc.tile_pool(name="yo", bufs=4) as yo_pool,
            tc.tile_pool(name="pmd", bufs=4, space="PSUM") as pmd,
            tc.tile_pool(name="pmu", bufs=3, space="PSUM") as pmu,
            tc.tile_pool(name="psc", bufs=1, space="PSUM") as pscp,
        ):
            # ---- x stream: 4 chunk-pair DMAs; weights interleaved early ----
            xt16 = []   # 4 tiles [128, 2, S] f16
            for j in range(DC // 2):
                t = xt_pool.tile([128, 2, S], F16, name=f"xt{j}", tag=f"xt{j}")
                nc.sync.dma_start(t[:], xT16[:, 2 * j:2 * j + 2, :])
                xt16.append(t)
                if j == 0:
                    wd_sb = w_pool.tile([128, DC, BOT], F8, tag="wd")
                    nc.sync.dma_start(wd_sb[:].bitcast(mybir.dt.uint8), wd8[:])

            wu_sb = w_pool.tile([128, OC, D], F8, tag="wu")
            nc.sync.dma_start(wu_sb[:].bitcast(mybir.dt.uint8), wu8[:])
            pq_sb = small.tile([128, 2 * OC], FP32)
            nc.sync.dma_start(pq_sb[:], pq[:])

            # h junk guard: up-proj rhs rows o>=400 multiply zero weights, but
            # stale SBUF bytes could be fp8 NaN/Inf; zero them once (gpsimd).
            ht = [ht_pool.tile([128, 2, S], F8, name=f"ht{j}", tag=f"ht{j}")
                  for j in range(2)]
            nc.gpsimd.memset(ht[1][:, 1, :], 0.0)

            # ---- downcast x -> fp8 (*32) on ACT, S1 via accum; S2 on DVE ----
            x8 = []     # 4 tiles [128, 2, S] f8
            sums = small.tile([128, DC], FP32)
            sqs = small.tile([128, DC], FP32)
            for j in range(DC // 2):
                t8 = x8_pool.tile([128, 2, S], F8, name=f"x8{j}", tag=f"x8{j}")
                for i in range(2):
                    c = 2 * j + i
                    if c < 6:
                        # ACT downcast; accum_out sums the scaled input -> S1
                        nc.scalar.activation(t8[:, i, :], xt16[j][:, i, :],
                                             mybir.ActivationFunctionType.Copy,
                                             scale=X_SCL,
                                             accum_out=sums[:, c:c + 1])
                    else:
                        # gpsimd downcast; S1 via DVE tensor_scalar (4x mode)
                        nc.gpsimd.tensor_scalar_mul(t8[:, i, :],
                                                    xt16[j][:, i, :], X_SCL)
                        sc1 = sq_pool.tile([128, S], F16, name=f"sc1_{c}", tag="sq")
                        nc.vector.tensor_scalar(sc1[:], xt16[j][:, i, :],
                                                X_SCL, 0.0,
                                                mybir.AluOpType.mult,
                                                mybir.AluOpType.add,
                                                accum_out=sums[:, c:c + 1])
                    # S2 on DVE: square via tensor_tensor (2x) then
                    # tensor_scalar accumulate (4x)
                    sq = sq_pool.tile([128, S], F16, name=f"sq_{c}", tag="sq")
                    nc.vector.tensor_tensor(sq[:], xt16[j][:, i, :],
                                            xt16[j][:, i, :],
                                            mybir.AluOpType.mult)
                    sq2 = sq_pool.tile([128, S], F16, name=f"sq2_{c}", tag="sq")
                    nc.vector.tensor_scalar(sq2[:], sq[:], 1.0, 0.0,
                                            mybir.AluOpType.mult,
                                            mybir.AluOpType.add,
                                            accum_out=sqs[:, c:c + 1])
                x8.append(t8)

            # ---- stats: mu, rstd, relu scale + bias ----
            s1 = small.tile([128, 1], FP32)
            s2 = small.tile([128, 1], FP32)
            nc.vector.tensor_reduce(s1[:], sums[:], mybir.AxisListType.X,
                                    mybir.AluOpType.add)
            nc.vector.tensor_reduce(s2[:], sqs[:], mybir.AxisListType.X,
                                    mybir.AluOpType.add)

            # inv1/inv2 are folded into the partition-reduce matmuls: the
            # "ones" columns carry the normalization constants, so psc[0,0]
            # = mu and psc[0,1] = E[x^2] directly.
            inv1_col = small.tile([128, 1], FP32)
            nc.vector.memset(inv1_col[:], inv1)
            inv2_col = small.tile([128, 1], FP32)
            nc.vector.memset(inv2_col[:], inv2)
            ones_row = small.tile([1, 128], FP32)
            nc.vector.memset(ones_row[:], 1.0)

            psc = pscp.tile([128, 8], FP32)
            nc.tensor.matmul(psc[0:1, 0:1], inv1_col[:], s1[:], start=True, stop=True)
            nc.tensor.matmul(psc[0:1, 1:2], inv2_col[:], s2[:], start=True, stop=True)

            sc = small.tile([1, 8], FP32)
            mu, e2, nvar, std, rstd, rs, mrn, mr = (sc[:, i:i + 1] for i in range(8))
            eps_sb = small.tile([1, 1], FP32)
            nc.vector.memset(eps_sb[:], EPS)
            nc.scalar.copy(sc[:, 0:2], psc[0:1, 0:2])
            # nvar = mu^2 - e2 ; std = sqrt(-nvar + eps) ; rstd = 1/std
            nc.vector.scalar_tensor_tensor(nvar, mu, mu, e2,
                                           mybir.AluOpType.mult,
                                           mybir.AluOpType.subtract)
            nc.scalar.activation(std, nvar, mybir.ActivationFunctionType.Sqrt,
                                 bias=eps_sb[:], scale=-1.0)
            nc.vector.reciprocal(rstd, std)
            # relu scale = rstd * H/(W*X); bias mult = -H * mu * rstd
            nc.vector.tensor_scalar_mul(rs, rstd, H_SCL * PSD_INV)
            nc.vector.tensor_tensor(mr, mu, rstd, mybir.AluOpType.mult)
            nc.vector.tensor_scalar_mul(mrn, mr, -H_SCL)

            # one matmul broadcasts both scalars across partitions
            nc.tensor.matmul(psc[:, 2:4], ones_row[:], sc[:, 5:7],
                             start=True, stop=True)
            bc = small.tile([128, 2], FP32)
            nc.scalar.copy(bc[:], psc[:, 2:4])

            # bias_sb[:, ot] = H*P[o] + (-H*mu*rstd) * Q[o]
            bias_sb = small.tile([128, OC], FP32)
            nc.vector.scalar_tensor_tensor(
                bias_sb[:], pq_sb[:, OC:2 * OC], bc[:, 1:2], pq_sb[:, 0:OC],
                mybir.AluOpType.mult, mybir.AluOpType.add)

            # ---- down-proj (fp8 DoubleRow) + ReLU -> h (fp8) ----
            # psum tile [128, 512] covers two 256-wide moving blocks.
            for sbp in range(NSBP):
                for ot in range(OC):
                    osz = O_SZ[ot]
                    ph = pmd.tile([128, 512], FP32, name=f"ph{ot}_{sbp}", tag="mmd")
                    for half in range(2):
                        sb = 2 * sbp + half
                        for kk in range(4):
                            nc.tensor.matmul(
                                ph[:osz, 256 * half:256 * (half + 1)],
                                wd_sb[:, 2 * kk:2 * kk + 2, 128 * ot:128 * ot + osz],
                                x8[kk][:, :, 256 * sb:256 * (sb + 1)],
                                start=(kk == 0), stop=(kk == 3), perf_mode=DR)
                    nc.scalar.activation(
                        ht[ot // 2][:osz, ot % 2, 512 * sbp:512 * (sbp + 1)],
                        ph[:osz, :],
                        mybir.ActivationFunctionType.Relu,
                        bias=bias_sb[:osz, ot:ot + 1], scale=bc[:osz, 0:1])

            # ---- up-proj (fp8 DoubleRow) + residual + store ----
            # Residual y = psum/(WU*H) + x, split across engines:
            #   dt 0-3: DVE scalar_tensor_tensor direct from PSUM (1x)
            #   dt 4-5: ACT scaled-evict to SBUF, then DVE tensor_tensor add (2x)
            #   dt 6-7: ACT scaled-evict to SBUF, then gpsimd tensor_tensor add
            for sbp in range(NSBP):
                yo = yo_pool.tile([128, DC, 512], F16, name=f"yo{sbp}", tag="yo")
                for dt in range(DC):
                    pu = pmu.tile([128, 512], FP32, name=f"pu{dt}_{sbp}", tag="mmu")
                    for half in range(2):
                        sb = 2 * sbp + half
                        for kk in range(2):
                            nc.tensor.matmul(
                                pu[:, 256 * half:256 * (half + 1)],
                                wu_sb[:, 2 * kk:2 * kk + 2, 128 * dt:128 * (dt + 1)],
                                ht[kk][:, :, 256 * sb:256 * (sb + 1)],
                                start=(kk == 0), stop=(kk == 1), perf_mode=DR)
                    xs = xt16[dt // 2][:, dt % 2, 512 * sbp:512 * (sbp + 1)]
                    if dt < 4:
                        nc.vector.scalar_tensor_tensor(
                            yo[:, dt, :], pu[:], PSU_INV, xs,
                            mybir.AluOpType.mult, mybir.AluOpType.add)
                    else:
                        nc.scalar.activation(yo[:, dt, :], pu[:],
                                             mybir.ActivationFunctionType.Copy,
                                             scale=PSU_INV)
                        eng = nc.vector if dt < 6 else nc.gpsimd
                        eng.tensor_tensor(yo[:, dt, :], yo[:, dt, :], xs,
                                          mybir.AluOpType.add)
                nc.sync.dma_start(yT[:, sbp, :, :], yo[:])

    nc.finalize()
    return nc


# ---------------------------------------------------------------------------
# Host-side orchestration
# ---------------------------------------------------------------------------

def prep_merge_inputs(alphas, W_down_all, W_up_all, W_ln_all, b_ln_all):
    """Build the 8 per-core input maps for the merge launch (fp16 stacks)."""
    a_in = np.ascontiguousarray(alphas.reshape(1, N)).astype(np.float32)
    wln = W_ln_all.reshape(N, DC, 128).transpose(0, 2, 1)   # [N,128,8]
    bln = b_ln_all.reshape(N, DC, 128).transpose(0, 2, 1)
    ln_blk = np.concatenate([wln, bln], axis=2)             # [N,128,16]
    in_maps = []
    for k in range(NCORES):
        wd_k = W_down_all[:, WD_ROWS * k:WD_ROWS * (k + 1), :].reshape(N, 128, 400)
        wu_k = W_up_all[:, WU_ROWS * k:WU_ROWS * (k + 1), :]  # [N,128,400]
        stack = np.concatenate([wd_k, wu_k, ln_blk], axis=2).astype(np.float16)
        in_maps.append({"stack": np.ascontiguousarray(stack), "alphas": a_in})
    return in_maps


def _to_f8(a):
    return np.clip(a, -F8_MAX, F8_MAX).astype(NP_F8)


def assemble_merge(results):
    """Per-core merge slices -> fp8 operands + PQ vectors for the main launch."""
    W_down = np.concatenate(
        [results[k]["out_m"][:, 0:400].astype(np.float32).reshape(WD_ROWS, D)
         for k in range(NCORES)], axis=0)                   # [BOT, D]
    W_up = np.concatenate(
        [results[k]["out_m"][:, 400:800].astype(np.float32)
         for k in range(NCORES)], axis=0)                   # [D, BOT]
    ln = results[0]["out_m"][:, 800:].astype(np.float32)    # [128, 16]
    W_ln = ln[:, 0:DC].T.reshape(D)
    b_ln = ln[:, DC:2 * DC].T.reshape(D)

    # wd8[p, c, o] = W_SCL * W_ln[128c+p] * W_down[o, 128c+p]
    wdT = W_down.T * (W_ln * W_SCL)[:, None]                # [D, BOT]
    wd8 = _to_f8(wdT.reshape(DC, 128, BOT).transpose(1, 0, 2))

    # wu8[p, c, d] = WU_SCL * W_up[d, 128c+p], rows o>=400 zero-padded
    wuT_pad = np.zeros((4 * 128, D), dtype=np.float32)
    wuT_pad[:BOT] = W_up.T * WU_SCL
    wu8 = _to_f8(wuT_pad.reshape(OC, 128, D).transpose(1, 0, 2))

    # PQ: cols 0:4 = H*P (o = 128c+p), cols 4:8 = Q
    P = W_down @ b_ln                                       # [BOT]
    Q = W_down @ W_ln                                       # [BOT]
    pq = np.zeros((128, 2 * OC), dtype=np.float32)
    Pp = np.zeros(512, dtype=np.float32); Pp[:BOT] = H_SCL * P
    Qp = np.zeros(512, dtype=np.float32); Qp[:BOT] = Q
    pq[:, 0:OC] = Pp.reshape(OC, 128).T
    pq[:, OC:2 * OC] = Qp.reshape(OC, 128).T
    return (np.ascontiguousarray(wd8).view(np.uint8),
            np.ascontiguousarray(wu8).view(np.uint8),
            np.ascontiguousarray(pq))


def prep_main_inputs(x, wd8, wu8, pq):
    in_maps = []
    for k in range(NCORES):
        xt = x[k].T.reshape(DC, 128, S).transpose(1, 0, 2).astype(np.float16)
        in_maps.append({"xT16": np.ascontiguousarray(xt),
                        "wd8": wd8, "wu8": wu8, "pq": pq})
    return in_maps


def assemble_output(results):
    out = np.empty((B, S, D), dtype=np.float32)
    for k in range(NCORES):
        y = results[k]["yT"].astype(np.float32)   # [128, NSBP, DC, 512]
        # element [p, sbp, dt, ss] = y[s=512*sbp+ss, d=128*dt+p]
        out[k] = y.transpose(1, 3, 2, 0).reshape(S, D)
    return out


_NC_CACHE = {}


def _get_nc(which):
    if which not in _NC_CACHE:
        _NC_CACHE[which] = build_merge_nc() if which == "merge" else build_main_nc()
    return _NC_CACHE[which]


def run(inputs, trace=False, trace_cores=None):
    """Run the full pipeline; returns (output, results_A, results_B)."""
    core_ids = list(range(NCORES))
    nc_a = _get_nc("merge")
    in_a = prep_merge_inputs(inputs["alphas"], inputs["W_down_all"],
                             inputs["W_up_all"], inputs["W_ln_all"],
                             inputs["b_ln_all"])
    res_a = run_bass_kernel_spmd(nc_a, in_a, core_ids=core_ids, trace=trace,
                                 trace_cores=trace_cores)
    wd8, wu8, pq = assemble_merge(res_a.results)

    nc_b = _get_nc("main")
    in_b = prep_main_inputs(inputs["x"], wd8, wu8, pq)
    res_b = run_bass_kernel_spmd(nc_b, in_b, core_ids=core_ids, trace=trace,
                                 trace_cores=trace_cores)
    out = assemble_output(res_b.results)
    return out, res_a, res_b


def kernel(**inputs):
    inputs = {k: np.asarray(v, dtype=np.float32) for k, v in inputs.items()}
    out, _, _ = run(inputs)
    return out


# revision 16
# speedup vs baseline: 1.2473x; 1.2473x over previous
"""MixAdapter: alpha-weighted adapter superposition + joint layernorm + bottleneck MLP.

Two SPMD launches on 8 NeuronCores (HW-calibrated engine assignment):

  Launch A ("merge"): fp16 adapter stacks sharded across cores (~5MB each).
    Scaled copies alpha_n*W_n run on ACT (20) and DVE (5); two parallel
    accumulation chains run on DVE (tensor_tensor, 2x mode) and gpsimd,
    combined at the end.  Host gathers the 0.8MB of merged params.

  Host folding (tiny): wdTw = W_ln*W_down scaled+quantized to fp8e4,
    wuT zero-padded/scaled/quantized, P/Q bias vectors.

  Launch B ("main"): batch elem k -> core k.
    - x^T fp16 in; ACT downcasts all 8 d-chunks to fp8 (x*32) with accum_out
      providing S1 (the sum).
    - S2: DVE squares x pairwise (tensor_tensor, 2x); PE ones-matmuls
      column-sum the squares into a PSUM accumulator; one small DVE reduce.
    - Down/up projections: fp8e4 DoubleRow matmuls with 1024-wide moving
      APs (512 output columns per instruction).
    - ReLU on ACT folds rstd/bias, requantizes h to fp8.
    - Residual y = psum/(WU*H) + x: dt 0-5 DVE stt from PSUM; dt 6-7 ACT
      scaled-evict + gpsimd add.  y written fp16, host upcasts.
"""

import numpy as np
import ml_dtypes

from concourse import bacc, mybir, tile
import concourse.bass as bass
from concourse.bass_utils import run_bass_kernel_spmd

B, S, D, BOT, N = 8, 2048, 1024, 400, 25
NCORES = 8
EPS = 1e-5
FP32 = mybir.dt.float32
F16 = mybir.dt.float16
F8 = mybir.dt.float8e4
U8 = mybir.dt.uint8
NP_F8 = ml_dtypes.float8_e4m3
F8_MAX = 240.0

DC = D // 128        # 8 d-chunks
OC = 4               # o-chunks (400 -> 3x128 + 16; padded to 512 for up-proj)
O_SZ = [128, 128, 128, 16]
NSBP = S // 512      # 4 psum-bank-wide moving groups

X_SCL = 32.0
W_SCL = 4096.0
WU_SCL = 1024.0
H_SCL = 64.0
PSD_INV = 1.0 / (W_SCL * X_SCL)
PSU_INV = 1.0 / (WU_SCL * H_SCL)

USE_F32R = False  # kept for test.py compatibility

WD_ROWS = BOT // NCORES
WU_ROWS = D // NCORES
MF = 400 + 400 + 2 * DC

DR = mybir.MatmulPerfMode.DoubleRow


# ---------------------------------------------------------------------------
# Launch A: alpha-weighted merge of the adapter stacks (sharded over cores)
# ---------------------------------------------------------------------------

N_ACT_COPY = 20   # adapters whose scaled copy runs on ACT (rest on DVE)
N_DVE_ACC = 16    # adapters 1..15 accumulate on DVE; 17.. on gpsimd


def build_merge_nc():
    nc = bacc.Bacc("TRN2", target_bir_lowering=False, debug=False,
                   enable_asserts=False, num_devices=NCORES)

    stack = nc.dram_tensor("stack", [N, 128, MF], F16, kind="ExternalInput")
    alphas = nc.dram_tensor("alphas", [1, N], FP32, kind="ExternalInput")
    out_m = nc.dram_tensor("out_m", [128, MF], F16, kind="ExternalOutput")

    with tile.TileContext(nc) as tc:
        with (
            tc.tile_pool(name="consts", bufs=1) as consts,
            tc.tile_pool(name="acc", bufs=1) as accp,
            tc.tile_pool(name="stk", bufs=8) as stk_pool,
            tc.tile_pool(name="tmp", bufs=6) as tmp_pool,
            tc.tile_pool(name="psum", bufs=1, space="PSUM") as psum,
        ):
            a_sb = consts.tile([1, N], FP32)
            nc.sync.dma_start(a_sb[:], alphas[:])
            ones_row = consts.tile([1, 128], FP32)
            nc.vector.memset(ones_row[:], 1.0)
            pa = psum.tile([128, N], FP32)
            nc.tensor.matmul(pa[:], ones_row[:], a_sb[:], start=True, stop=True)
            a_bc = consts.tile([128, N], FP32)
            nc.scalar.copy(a_bc[:], pa[:])

            acc_d = accp.tile([128, MF], F16)   # DVE chain: adapters 0..15
            acc_g = accp.tile([128, MF], F16)   # gpsimd chain: adapters 16..24
            for n in range(N):
                st = stk_pool.tile([128, MF], F16, name=f"st{n}", tag="st")
                nc.sync.dma_start(st[:], stack[n])
                al = a_bc[:, n:n + 1]
                # target of the scaled copy
                if n == 0:
                    dst = acc_d
                elif n == N_DVE_ACC:
                    dst = acc_g
                else:
                    dst = tmp_pool.tile([128, MF], F16, name=f"tm{n}", tag="tm")
                if n < N_ACT_COPY:
                    nc.scalar.activation(dst[:], st[:],
                                         mybir.ActivationFunctionType.Copy,
                                         scale=al)
                else:
                    nc.vector.tensor_scalar_mul(dst[:], st[:], al)
                if n in (0, N_DVE_ACC):
                    continue
                if n < N_DVE_ACC:
                    nc.vector.tensor_tensor(acc_d[:], acc_d[:], dst[:],
                                            mybir.AluOpType.add)
                else:
                    nc.gpsimd.tensor_tensor(acc_g[:], acc_g[:], dst[:],
                                            mybir.AluOpType.add)

            acc = accp.tile([128, MF], F16)
            nc.vector.tensor_tensor(acc[:], acc_d[:], acc_g[:],
                                    mybir.AluOpType.add)
            nc.sync.dma_start(out_m[:], acc[:])

    nc.finalize()
    return nc


# ---------------------------------------------------------------------------
# Launch B: layernorm + down/up projections, one batch element per core
# ---------------------------------------------------------------------------

def build_main_nc():
    nc = bacc.Bacc("TRN2", target_bir_lowering=False, debug=False,
                   enable_asserts=False, num_devices=NCORES)

    xT16 = nc.dram_tensor("xT16", [128, DC, S], F16, kind="ExternalInput")
    wd8 = nc.dram_tensor("wd8", [128, DC, BOT], U8, kind="ExternalInput")
    wu8 = nc.dram_tensor("wu8", [128, OC, D], U8, kind="ExternalInput")
    pq = nc.dram_tensor("pq", [128, 2 * OC], FP32, kind="ExternalInput")
    yT = nc.dram_tensor("yT", [128, NSBP, DC, 512], F16, kind="ExternalOutput")

    inv1 = 1.0 / (X_SCL * float(S * D))   # S1 -> mu
    inv2 = 1.0 / float(S * D)             # S2 -> E[x^2]

    with tile.TileContext(nc) as tc:
        with (
            tc.tile_pool(name="xt", bufs=1) as xt_pool,
            tc.tile_pool(name="x8", bufs=1) as x8_pool,
            tc.tile_pool(name="ht", bufs=1) as ht_pool,
            tc.tile_pool(name="w", bufs=1) as w_pool,
            tc.tile_pool(name="small", bufs=1) as small,
            tc.tile_pool(name="sq", bufs=3) as sq_pool,
            tc.tile_pool(name="yo", bufs=4) as yo_pool,
            tc.tile_pool(name="pmd", bufs=3, space="PSUM") as pmd,
            tc.tile_pool(name="pmu", bufs=3, space="PSUM") as pmu,
            tc.tile_pool(name="pst", bufs=1, space="PSUM") as pstp,
            tc.tile_pool(name="psc", bufs=1, space="PSUM") as pscp,
        ):
            # ---- x stream: 4 chunk-pair DMAs; weights interleaved ----
            xt16 = []
            for j in range(DC // 2):
                t = xt_pool.tile([128, 2, S], F16, name=f"xt{j}", tag=f"xt{j}")
                nc.sync.dma_start(t[:], xT16[:, 2 * j:2 * j + 2, :])
                xt16.append(t)
                if j == 0:
                    wd_sb = w_pool.tile([128, DC, BOT], F8, tag="wd")
                    nc.sync.dma_start(wd_sb[:].bitcast(U8), wd8[:])

            wu_sb = w_pool.tile([128, OC, D], F8, tag="wu")
            nc.sync.dma_start(wu_sb[:].bitcast(U8), wu8[:])
            pq_sb = small.tile([128, 2 * OC], FP32)
            nc.sync.dma_start(pq_sb[:], pq[:])

            ht = [ht_pool.tile([128, 2, S], F8, name=f"ht{j}", tag=f"ht{j}")
                  for j in range(2)]
            nc.gpsimd.memset(ht[1][:, 1, :], 0.0)

            ones16 = small.tile([128, 1], F16)
            nc.vector.memset(ones16[:], 1.0)

            # stats PSUM accumulator for column sums of x^2
            stat_ps = pstp.tile([1, 512], FP32)

            # ---- downcast (ACT, S1 via accum) + squares (DVE) + colsums (PE)
            x8 = []
            sums = small.tile([128, DC], FP32)
            for j in range(DC // 2):
                t8 = x8_pool.tile([128, 2, S], F8, name=f"x8{j}", tag=f"x8{j}")
                for i in range(2):
                    c = 2 * j + i
                    nc.scalar.activation(t8[:, i, :], xt16[j][:, i, :],
                                         mybir.ActivationFunctionType.Copy,
                                         scale=X_SCL,
                                         accum_out=sums[:, c:c + 1])
                sq = sq_pool.tile([128, 2, S], F16, name=f"sq{j}", tag="sq")
                nc.vector.tensor_tensor(sq[:], xt16[j][:], xt16[j][:],
                                        mybir.AluOpType.mult)
                for m in range(8):
                    nc.tensor.matmul(stat_ps[:],
                                     ones16[:],
                                     sq[:, m // 4, 512 * (m % 4):512 * (m % 4 + 1)],
                                     start=(j == 0 and m == 0),
                                     stop=(j == 3 and m == 7))
                x8.append(t8)

            # ---- stats scalar chain ----
            s1 = small.tile([128, 1], FP32)
            nc.vector.tensor_reduce(s1[:], sums[:], mybir.AxisListType.X,
                                    mybir.AluOpType.add)
            inv1_col = small.tile([128, 1], FP32)
            nc.vector.memset(inv1_col[:], inv1)
            ones_row = small.tile([1, 128], FP32)
            nc.vector.memset(ones_row[:], 1.0)

            psc = pscp.tile([128, 8], FP32)
            nc.tensor.matmul(psc[0:1, 0:1], inv1_col[:], s1[:], start=True, stop=True)

            sc = small.tile([1, 8], FP32)
            mu, s2r, e2, nvar, std, rstd, rs, mrn = (sc[:, i:i + 1] for i in range(8))
            mr = small.tile([1, 1], FP32)
            eps_sb = small.tile([1, 1], FP32)
            nc.vector.memset(eps_sb[:], EPS)
            nc.scalar.copy(mu, psc[0:1, 0:1])
            nc.vector.tensor_reduce(s2r, stat_ps[:], mybir.AxisListType.X,
                                    mybir.AluOpType.add)
            nc.vector.tensor_scalar_mul(e2, s2r, inv2)
            # nvar = mu^2 - e2 ; std = sqrt(-nvar + eps) ; rstd = 1/std
            nc.vector.scalar_tensor_tensor(nvar, mu, mu, e2,
                                           mybir.AluOpType.mult,
                                           mybir.AluOpType.subtract)
            nc.scalar.activation(std, nvar, mybir.ActivationFunctionType.Sqrt,
                                 bias=eps_sb[:], scale=-1.0)
            nc.vector.reciprocal(rstd, std)
            nc.vector.tensor_scalar_mul(rs, rstd, H_SCL * PSD_INV)
            nc.vector.tensor_tensor(mr, mu, rstd, mybir.AluOpType.mult)
            nc.vector.tensor_scalar_mul(mrn, mr, -H_SCL)

            nc.tensor.matmul(psc[:, 2:4], ones_row[:], sc[:, 6:8],
                             start=True, stop=True)
            bc = small.tile([128, 2], FP32)
            nc.scalar.copy(bc[:], psc[:, 2:4])

            bias_sb = small.tile([128, OC], FP32)
            nc.vector.scalar_tensor_tensor(
                bias_sb[:], pq_sb[:, OC:2 * OC], bc[:, 1:2], pq_sb[:, 0:OC],
                mybir.AluOpType.mult, mybir.AluOpType.add)

            # ---- down-proj (fp8 DoubleRow, 1024-wide moving) + ReLU ----
            for ot in range(OC):
                osz = O_SZ[ot]
                for sbp in range(NSBP):
                    ph = pmd.tile([128, 512], FP32, name=f"ph{ot}_{sbp}", tag="mmd")
                    for kk in range(4):
                        nc.tensor.matmul(
                            ph[:osz, :],
                            wd_sb[:, 2 * kk:2 * kk + 2, 128 * ot:128 * ot + osz],
                            x8[kk][:, :, 512 * sbp:512 * (sbp + 1)],
                            start=(kk == 0), stop=(kk == 3), perf_mode=DR)
                    nc.scalar.activation(
                        ht[ot // 2][:osz, ot % 2, 512 * sbp:512 * (sbp + 1)],
                        ph[:osz, :],
                        mybir.ActivationFunctionType.Relu,
                        bias=bias_sb[:osz, ot:ot + 1], scale=bc[:osz, 0:1])

            # ---- up-proj (fp8 DoubleRow) + residual + store ----
            for sbp in range(NSBP):
                yo = yo_pool.tile([128, DC, 512], F16, name=f"yo{sbp}", tag="yo")
                for dt in range(DC):
                    pu = pmu.tile([128, 512], FP32, name=f"pu{dt}_{sbp}", tag="mmu")
                    for kk in range(2):
                        nc.tensor.matmul(
                            pu[:],
                            wu_sb[:, 2 * kk:2 * kk + 2, 128 * dt:128 * (dt + 1)],
                            ht[kk][:, :, 512 * sbp:512 * (sbp + 1)],
                            start=(kk == 0), stop=(kk == 1), perf_mode=DR)
                    xs = xt16[dt // 2][:, dt % 2, 512 * sbp:512 * (sbp + 1)]
                    if dt < 6:
                        nc.vector.scalar_tensor_tensor(
                            yo[:, dt, :], pu[:], PSU_INV, xs,
                            mybir.AluOpType.mult, mybir.AluOpType.add)
                    else:
                        nc.scalar.activation(yo[:, dt, :], pu[:],
                                             mybir.ActivationFunctionType.Copy,
                                             scale=PSU_INV)
                        nc.gpsimd.tensor_tensor(yo[:, dt, :], yo[:, dt, :], xs,
                                                mybir.AluOpType.add)
                nc.sync.dma_start(yT[:, sbp, :, :], yo[:])

    nc.finalize()
    return nc


# ---------------------------------------------------------------------------
# Host-side orchestration
# ---------------------------------------------------------------------------

def prep_merge_inputs(alphas, W_down_all, W_up_all, W_ln_all, b_ln_all):
    a_in = np.ascontiguousarray(alphas.reshape(1, N)).astype(np.float32)
    wln = W_ln_all.reshape(N, DC, 128).transpose(0, 2, 1)
    bln = b_ln_all.reshape(N, DC, 128).transpose(0, 2, 1)
    ln_blk = np.concatenate([wln, bln], axis=2)             # [N,128,16]
    in_maps = []
    for k in range(NCORES):
        wd_k = W_down_all[:, WD_ROWS * k:WD_ROWS * (k + 1), :].reshape(N, 128, 400)
        wu_k = W_up_all[:, WU_ROWS * k:WU_ROWS * (k + 1), :]
        stack = np.concatenate([wd_k, wu_k, ln_blk], axis=2).astype(np.float16)
        in_maps.append({"stack": np.ascontiguousarray(stack), "alphas": a_in})
    return in_maps


def _to_f8(a):
    return np.clip(a, -F8_MAX, F8_MAX).astype(NP_F8)


def assemble_merge(results):
    W_down = np.concatenate(
        [results[k]["out_m"][:, 0:400].astype(np.float32).reshape(WD_ROWS, D)
         for k in range(NCORES)], axis=0)                   # [BOT, D]
    W_up = np.concatenate(
        [results[k]["out_m"][:, 400:800].astype(np.float32)
         for k in range(NCORES)], axis=0)                   # [D, BOT]
    ln = results[0]["out_m"][:, 800:].astype(np.float32)
    W_ln = ln[:, 0:DC].T.reshape(D)
    b_ln = ln[:, DC:2 * DC].T.reshape(D)

    wdT = W_down.T * (W_ln * W_SCL)[:, None]
    wd8 = _to_f8(wdT.reshape(DC, 128, BOT).transpose(1, 0, 2))

    wuT_pad = np.zeros((4 * 128, D), dtype=np.float32)
    wuT_pad[:BOT] = W_up.T * WU_SCL
    wu8 = _to_f8(wuT_pad.reshape(OC, 128, D).transpose(1, 0, 2))

    P = W_down @ b_ln
    Q = W_down @ W_ln
    pq = np.zeros((128, 2 * OC), dtype=np.float32)
    Pp = np.zeros(512, dtype=np.float32); Pp[:BOT] = H_SCL * P
    Qp = np.zeros(512, dtype=np.float32); Qp[:BOT] = Q
    pq[:, 0:OC] = Pp.reshape(OC, 128).T
    pq[:, OC:2 * OC] = Qp.reshape(OC, 128).T
    return (np.ascontiguousarray(wd8).view(np.uint8),
            np.ascontiguousarray(wu8).view(np.uint8),
            np.ascontiguousarray(pq))


def prep_main_inputs(x, wd8, wu8, pq):
    in_maps = []
    for k in range(NCORES):
        xt = x[k].T.reshape(DC, 128, S).transpose(1, 0, 2).astype(np.float16)
        in_maps.append({"xT16": np.ascontiguousarray(xt),
                        "wd8": wd8, "wu8": wu8, "pq": pq})
    return in_maps


def assemble_output(results):
    out = np.empty((B, S, D), dtype=np.float32)
    for k in range(NCORES):
        y = results[k]["yT"].astype(np.float32)   # [128, NSBP, DC, 512]
        out[k] = y.transpose(1, 3, 2, 0).reshape(S, D)
    return out


_NC_CACHE = {}


def _get_nc(which):
    if which not in _NC_CACHE:
        _NC_CACHE[which] = build_merge_nc() if which == "merge" else build_main_nc()
    return _NC_CACHE[which]


def run(inputs, trace=False, trace_cores=None):
    core_ids = list(range(NCORES))
    nc_a = _get_nc("merge")
    in_a = prep_merge_inputs(inputs["alphas"], inputs["W_down_all"],
                             inputs["W_up_all"], inputs["W_ln_all"],
                             inputs["b_ln_all"])
    res_a = run_bass_kernel_spmd(nc_a, in_a, core_ids=core_ids, trace=trace,
                                 trace_cores=trace_cores)
    wd8, wu8, pq = assemble_merge(res_a.results)

    nc_b = _get_nc("main")
    in_b = prep_main_inputs(inputs["x"], wd8, wu8, pq)
    res_b = run_bass_kernel_spmd(nc_b, in_b, core_ids=core_ids, trace=trace,
                                 trace_cores=trace_cores)
    out = assemble_output(res_b.results)
    return out, res_a, res_b


def kernel(**inputs):
    inputs = {k: np.asarray(v, dtype=np.float32) for k, v in inputs.items()}
    out, _, _ = run(inputs)
    return out


# revision 19
# speedup vs baseline: 1.5579x; 1.2490x over previous
"""MixAdapter: alpha-weighted adapter superposition + joint layernorm + bottleneck MLP.

Two SPMD launches on 8 NeuronCores (HW-calibrated engine assignment):

  Launch A ("merge"): fp16 adapter stacks sharded across cores (~5MB each).
    Scaled copies alpha_n*W_n run on ACT (20) and DVE (5); two parallel
    accumulation chains run on DVE (tensor_tensor, 2x mode) and gpsimd,
    combined at the end.  Host gathers the 0.8MB of merged params.

  Host folding (tiny): wdTw = W_ln*W_down scaled+quantized to fp8e4,
    wuT zero-padded/scaled/quantized, P/Q bias vectors.

  Launch B ("main"): batch elem k -> core k.
    - x^T fp16 in; ACT downcasts all 8 d-chunks to fp8 (x*32) with accum_out
      providing S1 (the sum).
    - S2: DVE squares x pairwise (tensor_tensor, 2x); PE ones-matmuls
      column-sum the squares into a PSUM accumulator; one small DVE reduce.
    - Down/up projections: fp8e4 DoubleRow matmuls with 1024-wide moving
      APs (512 output columns per instruction).
    - ReLU on ACT folds rstd/bias, requantizes h to fp8.
    - Residual y = psum/(WU*H) + x: dt 0-5 DVE stt from PSUM; dt 6-7 ACT
      scaled-evict + gpsimd add.  y written fp16, host upcasts.
"""

import numpy as np
import ml_dtypes

from concourse import bacc, mybir, tile
import concourse.bass as bass
from concourse.bass_utils import run_bass_kernel_spmd

B, S, D, BOT, N = 8, 2048, 1024, 400, 25
NCORES = 8
EPS = 1e-5
FP32 = mybir.dt.float32
F16 = mybir.dt.float16
F8 = mybir.dt.float8e4
U8 = mybir.dt.uint8
NP_F8 = ml_dtypes.float8_e4m3
F8_MAX = 240.0

DC = D // 128        # 8 d-chunks
OC = 4               # o-chunks (400 -> 3x128 + 16; padded to 512 for up-proj)
O_SZ = [128, 128, 128, 16]
NSBP = S // 512      # 4 psum-bank-wide moving groups

X_SCL = 32.0
W_SCL = 4096.0
WU_SCL = 1024.0
H_SCL = 64.0
PSD_INV = 1.0 / (W_SCL * X_SCL)
PSU_INV = 1.0 / (WU_SCL * H_SCL)

USE_F32R = False  # kept for test.py compatibility

WD_ROWS = BOT // NCORES
WU_ROWS = D // NCORES
MF = 400 + 400 + 2 * DC

DR = mybir.MatmulPerfMode.DoubleRow


# ---------------------------------------------------------------------------
# Launch A: alpha-weighted merge of the adapter stacks (sharded over cores)
# ---------------------------------------------------------------------------

N_ACT_COPY = 20   # adapters whose scaled copy runs on ACT (rest on DVE)
N_DVE_ACC = 16    # adapters 1..15 accumulate on DVE; 17.. on gpsimd


def build_merge_nc():
    nc = bacc.Bacc("TRN2", target_bir_lowering=False, debug=False,
                   enable_asserts=False, num_devices=NCORES)

    GRP = 5  # adapters per stack DMA
    stack = nc.dram_tensor("stack", [N // GRP, 128, GRP * MF], F16,
                           kind="ExternalInput")
    alphas = nc.dram_tensor("alphas", [1, N], FP32, kind="ExternalInput")
    out_m = nc.dram_tensor("out_m", [128, MF], F16, kind="ExternalOutput")

    with tile.TileContext(nc) as tc:
        with (
            tc.tile_pool(name="consts", bufs=1) as consts,
            tc.tile_pool(name="acc", bufs=1) as accp,
            tc.tile_pool(name="stk", bufs=3) as stk_pool,
            tc.tile_pool(name="psum", bufs=1, space="PSUM") as psum,
        ):
            a_sb = consts.tile([1, N], FP32)
            nc.sync.dma_start(a_sb[:], alphas[:])
            ones_row = consts.tile([1, 128], FP32)
            nc.vector.memset(ones_row[:], 1.0)
            pa = psum.tile([128, N], FP32)
            nc.tensor.matmul(pa[:], ones_row[:], a_sb[:], start=True, stop=True)
            a_bc = consts.tile([128, N], FP32)
            nc.scalar.copy(a_bc[:], pa[:])

            # single fused copy+scale+add chain on DVE (one op per adapter)
            acc = accp.tile([128, MF], F16)
            for g in range(N // GRP):
                st = stk_pool.tile([128, GRP, MF], F16, name=f"st{g}", tag="st")
                nc.sync.dma_start(st[:], stack[g])
                for q in range(GRP):
                    n = GRP * g + q
                    al = a_bc[:, n:n + 1]
                    if n == 0:
                        nc.vector.tensor_scalar_mul(acc[:], st[:, q, :], al)
                    else:
                        nc.vector.scalar_tensor_tensor(
                            acc[:], st[:, q, :], al, acc[:],
                            mybir.AluOpType.mult, mybir.AluOpType.add)

            nc.sync.dma_start(out_m[:], acc[:])

    nc.finalize()
    return nc


# ---------------------------------------------------------------------------
# Launch B: layernorm + down/up projections, one batch element per core
# ---------------------------------------------------------------------------

def build_main_nc():
    nc = bacc.Bacc("TRN2", target_bir_lowering=False, debug=False,
                   enable_asserts=False, num_devices=NCORES)

    xT16 = nc.dram_tensor("xT16", [128, DC, S], F16, kind="ExternalInput")
    wd8 = nc.dram_tensor("wd8", [128, DC, BOT], U8, kind="ExternalInput")
    wu8 = nc.dram_tensor("wu8", [128, OC, D], U8, kind="ExternalInput")
    pq = nc.dram_tensor("pq", [128, 2 * OC], FP32, kind="ExternalInput")
    yT = nc.dram_tensor("yT", [128, NSBP, DC, 512], F16, kind="ExternalOutput")

    inv1 = 1.0 / (X_SCL * float(S * D))   # S1 -> mu
    inv2 = 1.0 / float(S * D)             # S2 -> E[x^2]

    with tile.TileContext(nc) as tc:
        with (
            tc.tile_pool(name="xt", bufs=1) as xt_pool,
            tc.tile_pool(name="x8", bufs=1) as x8_pool,
            tc.tile_pool(name="ht", bufs=1) as ht_pool,
            tc.tile_pool(name="w", bufs=1) as w_pool,
            tc.tile_pool(name="small", bufs=1) as small,
            tc.tile_pool(name="sq", bufs=3) as sq_pool,
            tc.tile_pool(name="yo", bufs=4) as yo_pool,
            tc.tile_pool(name="pmd", bufs=2, space="PSUM") as pmd,
            tc.tile_pool(name="pmu", bufs=3, space="PSUM") as pmu,
            tc.tile_pool(name="pst", bufs=1, space="PSUM") as pstp,
        ):
            # ---- x stream: 4 chunk-pair DMAs; weights interleaved ----
            xt16 = []
            for j in range(DC // 2):
                t = xt_pool.tile([128, 2, S], F16, name=f"xt{j}", tag=f"xt{j}")
                nc.sync.dma_start(t[:], xT16[:, 2 * j:2 * j + 2, :])
                xt16.append(t)

            wd_sb = w_pool.tile([128, DC, BOT], F8, tag="wd")
            nc.sync.dma_start(wd_sb[:].bitcast(U8), wd8[:])
            wu_sb = w_pool.tile([128, OC, D], F8, tag="wu")
            nc.sync.dma_start(wu_sb[:].bitcast(U8), wu8[:])
            pq_sb = small.tile([128, 2 * OC], FP32)
            nc.sync.dma_start(pq_sb[:], pq[:])

            ht = [ht_pool.tile([128, 2, S], F8, name=f"ht{j}", tag=f"ht{j}")
                  for j in range(2)]
            nc.gpsimd.memset(ht[1][:, 1, :], 0.0)

            ones16 = small.tile([128, 1], F16)
            nc.vector.memset(ones16[:], 1.0)

            # stats PSUM bank: colsums of x^2 on partition 0, scalar matmul
            # outputs parked at other partitions/columns of the same bank
            pstc = pstp.tile([128, 512], FP32)
            stat_ps = pstc[0:1, 0:512]

            # ---- downcast (ACT, S1 via accum) + squares (DVE) + colsums (PE)
            x8 = []
            sums = small.tile([128, DC], FP32)
            for j in range(DC // 2):
                t8 = x8_pool.tile([128, 2, S], F8, name=f"x8{j}", tag=f"x8{j}")
                for i in range(2):
                    c = 2 * j + i
                    if c < 7:
                        nc.scalar.activation(t8[:, i, :], xt16[j][:, i, :],
                                             mybir.ActivationFunctionType.Copy,
                                             scale=X_SCL,
                                             accum_out=sums[:, c:c + 1])
                sq = sq_pool.tile([128, 2, S], F16, name=f"sq{j}", tag="sq")
                nc.vector.tensor_tensor(sq[:], xt16[j][:], xt16[j][:],
                                        mybir.AluOpType.mult)
                for m in range(8):
                    nc.tensor.matmul(stat_ps[:],
                                     ones16[:],
                                     sq[:, m // 4, 512 * (m % 4):512 * (m % 4 + 1)],
                                     start=(j == 0 and m == 0),
                                     stop=(j == 3 and m == 7))
                x8.append(t8)
            # last chunk downcast on DVE (fp8 out, 1x) with fused S1 accум
            nc.vector.tensor_scalar(x8[3][:, 1, :], xt16[3][:, 1, :],
                                    X_SCL, 0.0,
                                    mybir.AluOpType.mult, mybir.AluOpType.add,
                                    accum_out=sums[:, 7:8])

            # ---- stats scalar chain ----
            s1 = small.tile([128, 1], FP32)
            nc.vector.tensor_reduce(s1[:], sums[:], mybir.AxisListType.X,
                                    mybir.AluOpType.add)
            inv1_col = small.tile([128, 1], FP32)
            nc.vector.memset(inv1_col[:], inv1)
            ones_row = small.tile([1, 128], FP32)
            nc.vector.memset(ones_row[:], 1.0)

            nc.tensor.matmul(pstc[32:33, 0:1], inv1_col[:], s1[:],
                             start=True, stop=True)

            sc = small.tile([1, 8], FP32)
            mu, s2r, e2, nvar, std, rstd, rs, mrn = (sc[:, i:i + 1] for i in range(8))
            mr = small.tile([1, 1], FP32)
            eps_sb = small.tile([1, 1], FP32)
            nc.vector.memset(eps_sb[:], EPS)
            nc.scalar.copy(mu, pstc[32:33, 0:1])
            nc.vector.tensor_reduce(s2r, stat_ps[:], mybir.AxisListType.X,
                                    mybir.AluOpType.add)
            nc.vector.tensor_scalar_mul(e2, s2r, inv2)
            # nvar = mu^2 - e2 ; std = sqrt(-nvar + eps) ; rstd = 1/std
            nc.vector.scalar_tensor_tensor(nvar, mu, mu, e2,
                                           mybir.AluOpType.mult,
                                           mybir.AluOpType.subtract)
            nc.scalar.activation(std, nvar, mybir.ActivationFunctionType.Sqrt,
                                 bias=eps_sb[:], scale=-1.0)
            nc.vector.reciprocal(rstd, std)
            nc.vector.tensor_scalar_mul(rs, rstd, H_SCL * PSD_INV)
            nc.vector.tensor_tensor(mr, mu, rstd, mybir.AluOpType.mult)
            nc.vector.tensor_scalar_mul(mrn, mr, -H_SCL)

            nc.tensor.matmul(pstc[:, 2:4], ones_row[:], sc[:, 6:8],
                             start=True, stop=True)
            bc = small.tile([128, 2], FP32)
            nc.scalar.copy(bc[:], pstc[:, 2:4])

            bias_sb = small.tile([128, OC], FP32)
            nc.vector.scalar_tensor_tensor(
                bias_sb[:], pq_sb[:, OC:2 * OC], bc[:, 1:2], pq_sb[:, 0:OC],
                mybir.AluOpType.mult, mybir.AluOpType.add)

            # ---- down-proj (fp8 DoubleRow, 1024-wide moving) + ReLU ----
            for ot in range(OC):
                osz = O_SZ[ot]
                for sbpp in range(NSBP // 2):
                    ph = pmd.tile([128, 1024], FP32, name=f"ph{ot}_{sbpp}", tag="mmd")
                    for half in range(2):
                        sbp = 2 * sbpp + half
                        for kk in range(4):
                            nc.tensor.matmul(
                                ph[:osz, 512 * half:512 * (half + 1)],
                                wd_sb[:, 2 * kk:2 * kk + 2, 128 * ot:128 * ot + osz],
                                x8[kk][:, :, 512 * sbp:512 * (sbp + 1)],
                                start=(kk == 0), stop=(kk == 3), perf_mode=DR)
                    nc.scalar.activation(
                        ht[ot // 2][:osz, ot % 2, 1024 * sbpp:1024 * (sbpp + 1)],
                        ph[:osz, :],
                        mybir.ActivationFunctionType.Relu,
                        bias=bias_sb[:osz, ot:ot + 1], scale=bc[:osz, 0:1])

            # ---- up-proj (fp8 DoubleRow) + residual + store ----
            for sbp in range(NSBP):
                yo = yo_pool.tile([128, DC, 512], F16, name=f"yo{sbp}", tag="yo")
                for dt in range(DC):
                    pu = pmu.tile([128, 512], FP32, name=f"pu{dt}_{sbp}", tag="mmu")
                    for kk in range(2):
                        nc.tensor.matmul(
                            pu[:],
                            wu_sb[:, 2 * kk:2 * kk + 2, 128 * dt:128 * (dt + 1)],
                            ht[kk][:, :, 512 * sbp:512 * (sbp + 1)],
                            start=(kk == 0), stop=(kk == 1), perf_mode=DR)
                    xs = xt16[dt // 2][:, dt % 2, 512 * sbp:512 * (sbp + 1)]
                    if dt < 6:
                        nc.vector.scalar_tensor_tensor(
                            yo[:, dt, :], pu[:], PSU_INV, xs,
                            mybir.AluOpType.mult, mybir.AluOpType.add)
                    else:
                        nc.scalar.activation(yo[:, dt, :], pu[:],
                                             mybir.ActivationFunctionType.Copy,
                                             scale=PSU_INV)
                        nc.gpsimd.tensor_tensor(yo[:, dt, :], yo[:, dt, :], xs,
                                                mybir.AluOpType.add)
                nc.sync.dma_start(yT[:, sbp, :, :], yo[:])

    nc.finalize()
    return nc


# ---------------------------------------------------------------------------
# Host-side orchestration
# ---------------------------------------------------------------------------

def prep_merge_inputs(alphas, W_down_all, W_up_all, W_ln_all, b_ln_all):
    a_in = np.ascontiguousarray(alphas.reshape(1, N)).astype(np.float32)
    wln = W_ln_all.reshape(N, DC, 128).transpose(0, 2, 1)
    bln = b_ln_all.reshape(N, DC, 128).transpose(0, 2, 1)
    ln_blk = np.concatenate([wln, bln], axis=2)             # [N,128,16]
    in_maps = []
    for k in range(NCORES):
        wd_k = W_down_all[:, WD_ROWS * k:WD_ROWS * (k + 1), :].reshape(N, 128, 400)
        wu_k = W_up_all[:, WU_ROWS * k:WU_ROWS * (k + 1), :]
        stack = np.concatenate([wd_k, wu_k, ln_blk], axis=2).astype(np.float16)
        # group 5 adapters side-by-side in the free dim per DMA
        stack = stack.reshape(5, 5, 128, MF).transpose(0, 2, 1, 3).reshape(
            5, 128, 5 * MF)
        in_maps.append({"stack": np.ascontiguousarray(stack), "alphas": a_in})
    return in_maps


def _to_f8(a):
    return np.clip(a, -F8_MAX, F8_MAX).astype(NP_F8)


def assemble_merge(results):
    W_down = np.concatenate(
        [results[k]["out_m"][:, 0:400].astype(np.float32).reshape(WD_ROWS, D)
         for k in range(NCORES)], axis=0)                   # [BOT, D]
    W_up = np.concatenate(
        [results[k]["out_m"][:, 400:800].astype(np.float32)
         for k in range(NCORES)], axis=0)                   # [D, BOT]
    ln = results[0]["out_m"][:, 800:].astype(np.float32)
    W_ln = ln[:, 0:DC].T.reshape(D)
    b_ln = ln[:, DC:2 * DC].T.reshape(D)

    wdT = W_down.T * (W_ln * W_SCL)[:, None]
    wd8 = _to_f8(wdT.reshape(DC, 128, BOT).transpose(1, 0, 2))

    wuT_pad = np.zeros((4 * 128, D), dtype=np.float32)
    wuT_pad[:BOT] = W_up.T * WU_SCL
    wu8 = _to_f8(wuT_pad.reshape(OC, 128, D).transpose(1, 0, 2))

    P = W_down @ b_ln
    Q = W_down @ W_ln
    pq = np.zeros((128, 2 * OC), dtype=np.float32)
    Pp = np.zeros(512, dtype=np.float32); Pp[:BOT] = H_SCL * P
    Qp = np.zeros(512, dtype=np.float32); Qp[:BOT] = Q
    pq[:, 0:OC] = Pp.reshape(OC, 128).T
    pq[:, OC:2 * OC] = Qp.reshape(OC, 128).T
    return (np.ascontiguousarray(wd8).view(np.uint8),
            np.ascontiguousarray(wu8).view(np.uint8),
            np.ascontiguousarray(pq))


def prep_main_inputs(x, wd8, wu8, pq):
    in_maps = []
    for k in range(NCORES):
        xt = x[k].T.reshape(DC, 128, S).transpose(1, 0, 2).astype(np.float16)
        in_maps.append({"xT16": np.ascontiguousarray(xt),
                        "wd8": wd8, "wu8": wu8, "pq": pq})
    return in_maps


def assemble_output(results):
    out = np.empty((B, S, D), dtype=np.float32)
    for k in range(NCORES):
        y = results[k]["yT"].astype(np.float32)   # [128, NSBP, DC, 512]
        out[k] = y.transpose(1, 3, 2, 0).reshape(S, D)
    return out


_NC_CACHE = {}


def _get_nc(which):
    if which not in _NC_CACHE:
        _NC_CACHE[which] = build_merge_nc() if which == "merge" else build_main_nc()
    return _NC_CACHE[which]


def run(inputs, trace=False, trace_cores=None):
    core_ids = list(range(NCORES))
    nc_a = _get_nc("merge")
    in_a = prep_merge_inputs(inputs["alphas"], inputs["W_down_all"],
                             inputs["W_up_all"], inputs["W_ln_all"],
                             inputs["b_ln_all"])
    res_a = run_bass_kernel_spmd(nc_a, in_a, core_ids=core_ids, trace=trace,
                                 trace_cores=trace_cores)
    wd8, wu8, pq = assemble_merge(res_a.results)

    nc_b = _get_nc("main")
    in_b = prep_main_inputs(inputs["x"], wd8, wu8, pq)
    res_b = run_bass_kernel_spmd(nc_b, in_b, core_ids=core_ids, trace=trace,
                                 trace_cores=trace_cores)
    out = assemble_output(res_b.results)
    return out, res_a, res_b


def kernel(**inputs):
    inputs = {k: np.asarray(v, dtype=np.float32) for k, v in inputs.items()}
    out, _, _ = run(inputs)
    return out
